# revision 2
# baseline (speedup 1.0000x reference)
"""AtomWiseInvariants (GNN message passing) on 8 TRN2 NeuronCores — v5.

Strategy: shard by destination node (core i owns nodes [i*N/8, (i+1)*N/8));
host routes edges to their destination core (argsort by dst + padding), so
cores are fully independent — no collectives.

v5 eliminates the per-tile one-hot build (the DVE wall in v2-v4): each
node's edges are padded to a multiple of SLOT=8 rows, nodes are grouped by
slot-class q=ceil(d/8) and packed so a 128-row edge tile holds 16//q nodes
of one class. The scatter matmul's rhs is then a tiny CONSTANT 0/1 matrix
per class (rows 8q*i..8q*(i+1) -> column i) and each tile writes a disjoint
column range of its node block's [C, <=512] PSUM accumulator with
start=stop=True — no PSUM accumulation chains, no DVE compare op at all.
The envelope is folded into the rbf stream host-side (bias row = env), so
msg = filt * x needs just one DVE 2x multiply per 8-tile pair.

Node columns inside a block are class-permuted; the MLP is elementwise
over columns so it runs on the permuted layout, and the host inverts the
permutation when assembling the output.

Remaining per-8-tile-pair device work: 2 bf16 filter matmuls (PSUM bank
pair), 1 ACT copy f32->bf16 (or DVE multiply straight from PSUM for a
fraction of pairs, to balance ACT vs DVE), 1 DVE 2x multiply, 8 tiny
scatter matmuls. DMA is supergrouped (16 tiles per dma_start pair).
"""

import math

import numpy as np

# ---------------------------------------------------------------- config

NCORES = 8
P = 128           # partitions / edge tile size
SLOT = 8          # slot quantum per node
C = 128
G = 4             # edge tiles per filter matmul group (one PSUM bank)
PAIR = 2 * G      # tiles per ACT/DVE processing pair (two PSUM banks)
SG = 4            # groups per DMA supergroup (16 tiles)
RBF_DIM = 20
RK = RBF_DIM + 1  # augmented contraction dim (bias row)
BCOLS = 512       # node columns per MLP block (one PSUM bank)
BGRAN = 32        # node-range granularity when packing blocks
LEADP = 2         # pair software-pipeline lead distance (filt vs scatter)
LEADS = 1         # supergroup DMA lead distance

# fp8(e4m3) filter matmul via DoubleRow (0.5 cyc/row): rbf+wabd in fp8,
# contraction 84 split 42+42 across the two row-sets
FP8_FILT = False

# CoreSim lacks Silu; True decomposes it as v*sigmoid(v) for sim runs
SILU_DECOMP = False

XSGW = SG * G * C          # xeg columns per supergroup row


def _bf16():
    import ml_dtypes
    return ml_dtypes.bfloat16


def _npt(q):
    """Nodes per 128-row tile for class q (8q rows per node)."""
    return P // (SLOT * q)


# ------------------------------------------------------------- host prep

def _schedule(deg_all):
    """Build the shared (SPMD) schedule from per-core node degrees.

    deg_all: [NCORES, npc] int. Returns (blocks, tile_sched) where
      blocks[b] = (node_lo, node_hi, cols_used)
      tile_sched[t] = (block, q, col_start)  in device execution order
    """
    npc = deg_all.shape[1]
    qcls = np.maximum(1, -(-deg_all // SLOT))          # [NCORES, npc]
    qmax = int(qcls.max())
    assert qmax * SLOT <= P, f"node degree {int(deg_all.max())} > {P}"

    # prefix counts per class for fast range queries
    pref = {}
    for q in range(1, qmax + 1):
        m = (qcls == q).astype(np.int64)
        pref[q] = np.concatenate(
            [np.zeros((NCORES, 1), np.int64), np.cumsum(m, axis=1)], axis=1)

    def cols_for(lo, hi):
        tot = 0
        per_q = {}
        for q in range(1, qmax + 1):
            cnt = pref[q][:, hi] - pref[q][:, lo]      # per core
            ntiles = int(np.max(-(-cnt // _npt(q))))
            if ntiles:
                per_q[q] = ntiles
                tot += ntiles * _npt(q)
        return tot, per_q

    blocks = []
    tile_sched = []
    lo = 0
    while lo < npc:
        hi = min(lo + BGRAN, npc)
        tot, per_q = cols_for(lo, hi)
        while hi < npc:
            nhi = min(hi + BGRAN, npc)
            t2, p2 = cols_for(lo, nhi)
            if t2 > BCOLS:
                break
            hi, tot, per_q = nhi, t2, p2
        b = len(blocks)
        cur = 0
        for q in sorted(per_q):
            for i in range(per_q[q]):
                tile_sched.append((b, q, cur))
                cur += _npt(q)
        blocks.append((lo, hi, tot))
        lo = hi
    return blocks, tile_sched


def _prep_core(x, rbf, envf, dst_local, deg, blocks, tile_sched, bf16):
    """Build one core's padded streams + node->output-position map.

    x/rbf/envf: this core's edges sorted by dst_local. envf folded: rbf
    columns are pre-multiplied by env and the bias row is env itself.
    Returns (xeg, rbg, node_pos) where node_pos[n] = flat output index.
    """
    npc = len(deg)
    TT = len(tile_sched)
    TSG = math.ceil(TT / (G * SG))
    TTg16 = TSG * SG * G

    qcls = np.maximum(1, -(-deg // SLOT))

    # tile index layout per (block, q): consecutive in schedule order
    tidx = {}
    for t, (b, q, cs) in enumerate(tile_sched):
        tidx.setdefault((b, q), []).append((t, cs))

    # block offsets in the output vector
    blk_off = np.zeros(len(blocks) + 1, dtype=np.int64)
    for b, (_, _, used) in enumerate(blocks):
        blk_off[b + 1] = blk_off[b] + used

    # assign each node a (tile, row0, col)
    node_tile = np.zeros(npc, dtype=np.int64)
    node_row0 = np.zeros(npc, dtype=np.int64)
    node_pos = np.zeros(npc, dtype=np.int64)
    for b, (nlo, nhi, _) in enumerate(blocks):
        ids = np.arange(nlo, nhi)
        for q in np.unique(qcls[nlo:nhi]):
            sel = ids[qcls[nlo:nhi] == q]
            npt = _npt(q)
            tiles = tidx[(b, int(q))]
            for j, n in enumerate(sel):
                ti, cs = tiles[j // npt]
                r = j % npt
                node_tile[n] = ti
                node_row0[n] = r * SLOT * q
                node_pos[n] = blk_off[b] + cs + r

    # per-edge slot
    starts = np.zeros(npc, dtype=np.int64)
    starts[1:] = np.cumsum(deg)[:-1]
    rank = np.arange(len(dst_local), dtype=np.int64) - starts[dst_local]
    slot = node_tile[dst_local] * P + node_row0[dst_local] + rank

    x_t = np.zeros((TTg16 * P, C), dtype=bf16)
    x_t[slot] = x
    rb_t = np.zeros((TTg16 * P, RK), dtype=bf16)
    rb_t[slot, :RBF_DIM] = rbf * envf[:, None]
    rb_t[slot, RBF_DIM] = envf

    xeg = np.zeros((TSG, P, XSGW), dtype=bf16)
    xt = x_t.reshape(TSG, SG * G, P, C)
    for u in range(SG * G):
        xeg[:, :, u * C:(u + 1) * C] = xt[:, u]
    rbg = (rb_t.reshape(TSG, SG, G, P, RK).transpose(0, 2, 4, 1, 3)
           .reshape(TSG, G * RK, SG * P))
    if FP8_FILT:
        import ml_dtypes
        fp8 = ml_dtypes.float8_e4m3fn
        # [TSG, 84, SG*P] -> [TSG, 42, SG*2P]: per group block [42, 2P]
        # cols [0:P] = rows 0:42 (tiles 0-1), cols [P:2P] = rows 42:84
        r = rbg.reshape(TSG, 2, G * RK // 2, SG, P)
        rbg = np.ascontiguousarray(
            r.transpose(0, 2, 3, 1, 4)).astype(fp8)
    return (np.ascontiguousarray(xeg), np.ascontiguousarray(rbg),
            node_pos, int(blk_off[-1]))


def prepare(x_scalar, rbf, envelop_para, edge_index_0, num_atoms,
            W_rbf, b_rbf, W1, b1, W2, b2, W3, b3):
    """Host-side sharding/layout. Returns (in_maps, meta)."""
    bf16 = _bf16()
    N = int(num_atoms)
    assert x_scalar.shape[1] == C
    assert N % NCORES == 0
    npc = N // NCORES

    dst = np.asarray(edge_index_0, dtype=np.int64)
    order = np.argsort(dst, kind="stable")
    dst_s = dst[order]
    x_s = np.asarray(x_scalar, dtype=np.float32)[order]
    rbf_s = np.asarray(rbf, dtype=np.float32)[order]
    env_s = np.asarray(envelop_para, dtype=np.float32).reshape(-1)[order]

    core_of = dst_s // npc
    core_bounds = np.searchsorted(core_of, np.arange(NCORES + 1))

    deg_all = np.zeros((NCORES, npc), dtype=np.int64)
    for c in range(NCORES):
        lo, hi = core_bounds[c], core_bounds[c + 1]
        deg_all[c] = np.bincount(dst_s[lo:hi] - c * npc, minlength=npc)

    blocks, tile_sched = _schedule(deg_all)

    # block-diagonal augmented filter weight [G*RK, G*C], bf16
    wa = np.zeros((RK, C), dtype=np.float32)
    wa[:RBF_DIM] = np.asarray(W_rbf, np.float32).T
    wa[RBF_DIM] = np.asarray(b_rbf, np.float32)
    wabd = np.zeros((G * RK, G * C), dtype=bf16)
    for j in range(G):
        wabd[j * RK:(j + 1) * RK, j * C:(j + 1) * C] = wa
    if FP8_FILT:
        import ml_dtypes
        fp8 = ml_dtypes.float8_e4m3fn
        wabd = np.stack(
            [wabd[:G * RK // 2], wabd[G * RK // 2:]], axis=1).astype(fp8)

    # constant per-class scatter matrices, packed into one [P, RCOLS]
    qs = sorted({q for _, q, _ in tile_sched})
    rq_off = {}
    cur = 0
    for q in qs:
        rq_off[q] = cur
        cur += _npt(q)
    rhs_all = np.zeros((P, cur), dtype=bf16)
    for q in qs:
        npt = _npt(q)
        for i in range(npt):
            rhs_all[i * SLOT * q:(i + 1) * SLOT * q, rq_off[q] + i] = 1.0

    consts = {
        "wabd": wabd,
        "rhs_all": rhs_all,
        "w1t": np.ascontiguousarray(np.asarray(W1, np.float32).T.astype(bf16)),
        "w2t": np.ascontiguousarray(np.asarray(W2, np.float32).T.astype(bf16)),
        "w3t": np.ascontiguousarray(np.asarray(W3, np.float32).T.astype(bf16)),
        "b1": np.asarray(b1, np.float32).reshape(C, 1),
        "b2": np.asarray(b2, np.float32).reshape(C, 1),
        "b3": np.asarray(b3, np.float32).reshape(1, 1),
    }
    in_maps = []
    perms = []
    out_len = None
    for c in range(NCORES):
        lo, hi = core_bounds[c], core_bounds[c + 1]
        xeg, rbg, node_pos, olen = _prep_core(
            x_s[lo:hi], rbf_s[lo:hi], env_s[lo:hi],
            dst_s[lo:hi] - c * npc, deg_all[c], blocks, tile_sched, bf16)
        in_maps.append({"xeg": xeg, "rbg": rbg, **consts})
        perms.append(node_pos)
        out_len = olen

    meta = dict(N=N, C=C, npc=npc, blocks=blocks, tile_sched=tile_sched,
                rq_off=rq_off, rhs_cols=cur, out_len=out_len, perms=perms)
    return in_maps, meta


# ----------------------------------------------------------- bass kernel

def build_graph(meta):
    import concourse.bacc as bacc
    import concourse.mybir as mybir
    import concourse.tile as tile

    f32 = mybir.dt.float32
    bf16 = mybir.dt.bfloat16
    AF = mybir.ActivationFunctionType
    OP = mybir.AluOpType

    blocks = meta["blocks"]
    tile_sched = meta["tile_sched"]
    rq_off = meta["rq_off"]
    RCOLS = meta["rhs_cols"]
    OUT_LEN = meta["out_len"]
    NB = len(blocks)
    TT = len(tile_sched)
    TSG = math.ceil(TT / (G * SG))
    NPAIR = math.ceil(TT / PAIR)

    blk_off = [0]
    for _, _, used in blocks:
        blk_off.append(blk_off[-1] + used)
    last_tile_of_block = {}
    for t, (b, _, _) in enumerate(tile_sched):
        last_tile_of_block[b] = t

    nc = bacc.Bacc(None, target_bir_lowering=False, debug=False)

    fp8 = mybir.dt.float8e4
    rb_dt = fp8 if FP8_FILT else bf16
    RBROWS = G * RK // 2 if FP8_FILT else G * RK

    xeg_d = nc.declare_dram_parameter("xeg", [TSG, P, XSGW], bf16,
                                      isOutput=False)
    if FP8_FILT:
        rbg_d = nc.declare_dram_parameter(
            "rbg", [TSG, RBROWS, SG, 2, P], rb_dt, isOutput=False)
        wabd_d = nc.declare_dram_parameter(
            "wabd", [RBROWS, 2, G * C], rb_dt, isOutput=False)
    else:
        rbg_d = nc.declare_dram_parameter(
            "rbg", [TSG, RBROWS, SG * P], rb_dt, isOutput=False)
        wabd_d = nc.declare_dram_parameter(
            "wabd", [RBROWS, G * C], rb_dt, isOutput=False)
    rhs_d = nc.declare_dram_parameter("rhs_all", [P, RCOLS], bf16,
                                      isOutput=False)
    w1t_d = nc.declare_dram_parameter("w1t", [C, C], bf16, isOutput=False)
    w2t_d = nc.declare_dram_parameter("w2t", [C, C], bf16, isOutput=False)
    w3t_d = nc.declare_dram_parameter("w3t", [C, 1], bf16, isOutput=False)
    b1_d = nc.declare_dram_parameter("b1", [C, 1], f32, isOutput=False)
    b2_d = nc.declare_dram_parameter("b2", [C, 1], f32, isOutput=False)
    b3_d = nc.declare_dram_parameter("b3", [1, 1], f32, isOutput=False)
    out_d = nc.declare_dram_parameter("out", [OUT_LEN], f32, isOutput=True)

    with tile.TileContext(nc) as tc:
        with (
            tc.tile_pool(name="const", bufs=1) as cp,
            tc.tile_pool(name="xin", bufs=LEADS + 3) as xp,
            tc.tile_pool(name="rin", bufs=LEADS + 3) as rp,
            tc.tile_pool(name="fc", bufs=3) as fcp,
            tc.tile_pool(name="msg", bufs=6) as mp,
            tc.tile_pool(name="mlp", bufs=2) as hp,
            tc.tile_pool(name="ys", bufs=2) as ysp,
            tc.tile_pool(name="fps", bufs=3, space="PSUM") as fps,
            tc.tile_pool(name="ops", bufs=2, space="PSUM") as ops,
        ):
            if FP8_FILT:
                wabd_s = cp.tile([RBROWS, 2, G * C], rb_dt)
                nc.sync.dma_start(out=wabd_s[:], in_=wabd_d[:, :, :])
            else:
                wabd_s = cp.tile([RBROWS, G * C], rb_dt)
                nc.sync.dma_start(out=wabd_s[:], in_=wabd_d[:, :])
            rhs_s = cp.tile([P, RCOLS], bf16)
            nc.sync.dma_start(out=rhs_s[:], in_=rhs_d[:, :])

            def load_mlp_consts():
                w1t_s = cp.tile([C, C], bf16)
                nc.sync.dma_start(out=w1t_s[:], in_=w1t_d[:, :])
                w2t_s = cp.tile([C, C], bf16)
                nc.sync.dma_start(out=w2t_s[:], in_=w2t_d[:, :])
                w3t_s = cp.tile([C, 1], bf16)
                nc.sync.dma_start(out=w3t_s[:], in_=w3t_d[:, :])
                b1_s = cp.tile([C, 1], f32)
                nc.sync.dma_start(out=b1_s[:], in_=b1_d[:, :])
                b2_s = cp.tile([C, 1], f32)
                nc.sync.dma_start(out=b2_s[:], in_=b2_d[:, :])
                b3_s = cp.tile([1, 1], f32)
                nc.sync.dma_start(out=b3_s[:], in_=b3_d[:, :])
                return w1t_s, w2t_s, w3t_s, b1_s, b2_s, b3_s

            xe_s, rb_s = {}, {}
            xe_s, rb_s = {}, {}
            msg_p = {}
            outT = {}
            pend_mlp = []

            def silu(dst_ap, src_ap, bias, wcols):
                if SILU_DECOMP:
                    z = hp.tile([C, BCOLS], f32, tag="sz")
                    nc.scalar.activation(z[:, :wcols], src_ap, AF.Identity,
                                         bias=bias[:])
                    s = hp.tile([C, BCOLS], f32, tag="ss")
                    nc.scalar.activation(s[:, :wcols], src_ap, AF.Sigmoid,
                                         bias=bias[:])
                    nc.vector.tensor_tensor(out=dst_ap, in0=z[:, :wcols],
                                            in1=s[:, :wcols], op=OP.mult)
                else:
                    nc.scalar.activation(dst_ap, src_ap, AF.Silu,
                                         bias=bias[:])

            def emit_mlp(b):
                wcols = blocks[b][2]
                o = outT.pop(b)
                a0 = hp.tile([C, BCOLS], bf16, tag="a0")
                nc.scalar.activation(a0[:, :wcols], o[:, :wcols], AF.Copy)
                h1p = ops.tile([C, BCOLS], f32, space="PSUM",
                               name="h1p", tag="outT")
                nc.tensor.matmul(out=h1p[:, :wcols], lhsT=w1t_s[:],
                                 rhs=a0[:, :wcols], start=True, stop=True)
                h1 = hp.tile([C, BCOLS], bf16, tag="h1")
                silu(h1[:, :wcols], h1p[:, :wcols], b1_s, wcols)
                h2p = ops.tile([C, BCOLS], f32, space="PSUM",
                               name="h2p", tag="outT")
                nc.tensor.matmul(out=h2p[:, :wcols], lhsT=w2t_s[:],
                                 rhs=h1[:, :wcols], start=True, stop=True)
                h2 = hp.tile([C, BCOLS], bf16, tag="h2")
                silu(h2[:, :wcols], h2p[:, :wcols], b2_s, wcols)
                yp = h2p[0:1, :]
                nc.tensor.matmul(out=yp[:, :wcols], lhsT=w3t_s[:],
                                 rhs=h2[:, :wcols], start=True, stop=True)
                ys = ysp.tile([1, BCOLS], f32, tag="ys")
                nc.scalar.activation(ys[:, :wcols], yp[:, :wcols],
                                     AF.Identity, bias=b3_s[:])
                nc.sync.dma_start(
                    out=out_d[None, blk_off[b]:blk_off[b] + wcols],
                    in_=ys[:, :wcols])

            def stage_load(s):
                xe = xp.tile([P, XSGW], bf16, tag="xe")
                nc.sync.dma_start(out=xe[:], in_=xeg_d[s, :, :])
                if FP8_FILT:
                    rb = rp.tile([RBROWS, SG, 2, P], rb_dt, tag="rb")
                    nc.sync.dma_start(out=rb[:], in_=rbg_d[s, :, :, :, :])
                else:
                    rb = rp.tile([RBROWS, SG * P], rb_dt, tag="rb")
                    nc.sync.dma_start(out=rb[:], in_=rbg_d[s, :, :])
                xe_s[s] = xe
                rb_s[s] = rb

            def stage_filt(pr):
                g0 = pr * 2
                sgi, q2 = divmod(g0, SG)
                xe = xe_s[sgi]
                rb = rb_s[sgi]
                fp2 = fps.tile([P, PAIR * C], f32, space="PSUM", tag="filt")
                for h in range(2):
                    if FP8_FILT:
                        nc.tensor.matmul(
                            out=fp2[:, h * G * C:(h + 1) * G * C],
                            lhsT=rb[:, q2 + h, :, :], rhs=wabd_s[:],
                            start=True, stop=True,
                            perf_mode=mybir.MatmulPerfMode.DoubleRow)
                    else:
                        nc.tensor.matmul(
                            out=fp2[:, h * G * C:(h + 1) * G * C],
                            lhsT=rb[:, (q2 + h) * P:(q2 + h + 1) * P],
                            rhs=wabd_s[:], start=True, stop=True)
                msg = mp.tile([P, PAIR * C], bf16, tag="msg")
                xblk = xe[:, q2 * G * C:(q2 + 2) * G * C]
                half = G * C
                # half 1 via ACT copy->bf16, half 2 via DVE straight from
                # PSUM — runs in parallel, halves the fp2 hold time
                fc = fcp.tile([P, half], bf16, tag="fc")
                nc.scalar.activation(fc[:], fp2[:, :half], AF.Copy)
                nc.vector.tensor_tensor(out=msg[:, half:],
                                        in0=fp2[:, half:],
                                        in1=xblk[:, half:], op=OP.mult)
                nc.vector.tensor_tensor(out=msg[:, :half], in0=fc[:],
                                        in1=xblk[:, :half], op=OP.mult)
                msg_p[pr] = msg

            def stage_scatter(pr):
                g0 = pr * 2
                msg = msg_p.pop(pr)
                for j in range(PAIR):
                    t = g0 * G + j
                    if t >= TT:
                        break
                    b, q, cs = tile_sched[t]
                    if b not in outT:
                        outT[b] = ops.tile([C, BCOLS], f32, space="PSUM",
                                           name="outT", tag="outT")
                    npt = _npt(q)
                    ro = rq_off[q]
                    nc.tensor.matmul(
                        out=outT[b][:, cs:cs + npt],
                        lhsT=msg[:, j * C:(j + 1) * C],
                        rhs=rhs_s[:, ro:ro + npt],
                        start=True, stop=True, skip_group_check=True)
                    if t == last_tile_of_block[b]:
                        pend_mlp.append(b)

            for si in range(min(LEADS + 1, TSG)):
                stage_load(si)
            w1t_s, w2t_s, w3t_s, b1_s, b2_s, b3_s = load_mlp_consts()
            NPAIR = math.ceil(TT / PAIR)
            for pr in range(NPAIR + LEADP):
                g0 = pr * 2
                if g0 % SG == 0:
                    s_next = g0 // SG + LEADS + 1
                    if s_next < TSG:
                        stage_load(s_next)
                if pr < NPAIR:
                    stage_filt(pr)
                if pr >= LEADP and (pr - LEADP) * PAIR < TT:
                    ps = pr - LEADP
                    stage_scatter(ps)
                    last_t = min((ps + 1) * PAIR, TT) - 1
                    cur_b = tile_sched[last_t][0]
                    while pend_mlp and (pend_mlp[0] < cur_b
                                        or (ps + 1) * PAIR >= TT):
                        emit_mlp(pend_mlp.pop(0))
            while pend_mlp:
                emit_mlp(pend_mlp.pop(0))

    nc.compile()
    return nc


# --------------------------------------------------------------- driver

def run(inputs, trace=False, tmpdir=None):
    from concourse.bass_utils import run_bass_kernel_spmd

    in_maps, meta = prepare(**inputs)
    nc = build_graph(meta)
    res = run_bass_kernel_spmd(nc, in_maps, core_ids=list(range(NCORES)),
                               trace=trace, tmpdir=tmpdir)
    npc = meta["npc"]
    outs = []
    for c in range(NCORES):
        flat = np.asarray(res.results[c]["out"])
        outs.append(flat[meta["perms"][c]])
    return np.concatenate(outs).reshape(meta["N"], 1).astype(np.float32), res


def kernel(**inputs):
    out, _ = run(inputs, trace=False)
    return out


# revision 3
# speedup vs baseline: 1.1537x; 1.1537x over previous
"""AtomWiseInvariants (GNN message passing) on 8 TRN2 NeuronCores — v5.

Strategy: shard by destination node (core i owns nodes [i*N/8, (i+1)*N/8));
host routes edges to their destination core (argsort by dst + padding), so
cores are fully independent — no collectives.

v5 eliminates the per-tile one-hot build (the DVE wall in v2-v4): each
node's edges are padded to a multiple of SLOT=8 rows, nodes are grouped by
slot-class q=ceil(d/8) and packed so a 128-row edge tile holds 16//q nodes
of one class. The scatter matmul's rhs is then a tiny CONSTANT 0/1 matrix
per class (rows 8q*i..8q*(i+1) -> column i) and each tile writes a disjoint
column range of its node block's [C, <=512] PSUM accumulator with
start=stop=True — no PSUM accumulation chains, no DVE compare op at all.
The envelope is folded into the rbf stream host-side (bias row = env), so
msg = filt * x needs just one DVE 2x multiply per 8-tile pair.

Node columns inside a block are class-permuted; the MLP is elementwise
over columns so it runs on the permuted layout, and the host inverts the
permutation when assembling the output.

Remaining per-8-tile-pair device work: 2 bf16 filter matmuls (PSUM bank
pair), 1 ACT copy f32->bf16 (or DVE multiply straight from PSUM for a
fraction of pairs, to balance ACT vs DVE), 1 DVE 2x multiply, 8 tiny
scatter matmuls. DMA is supergrouped (16 tiles per dma_start pair).
"""

import math

import numpy as np

# ---------------------------------------------------------------- config

NCORES = 8
P = 128           # partitions / edge tile size
SLOT = 8          # slot quantum per node
C = 128
G = 4             # edge tiles per filter matmul group (one PSUM bank)
PAIR = 2 * G      # tiles per ACT/DVE processing pair (two PSUM banks)
SG = 4            # groups per DMA supergroup (16 tiles)
RBF_DIM = 20
RK = RBF_DIM + 1  # augmented contraction dim (bias row)
BCOLS = 512       # node columns per MLP block (one PSUM bank)
BGRAN = 32        # node-range granularity when packing blocks
LEADP = 2         # pair software-pipeline lead distance (filt vs scatter)
LEADS = 1         # supergroup DMA lead distance

# fp8(e4m3) filter matmul via DoubleRow (0.5 cyc/row): rbf+wabd in fp8,
# contraction 84 split 42+42 across the two row-sets
FP8_FILT = False

# CoreSim lacks Silu; True decomposes it as v*sigmoid(v) for sim runs
SILU_DECOMP = False

XSGW = SG * G * C          # xeg columns per supergroup row


def _bf16():
    import ml_dtypes
    return ml_dtypes.bfloat16


def _npt(q):
    """Nodes per 128-row tile for class q (8q rows per node)."""
    return P // (SLOT * q)


# ------------------------------------------------------------- host prep

def _schedule(deg_all):
    """Build the shared (SPMD) schedule from per-core node degrees.

    deg_all: [NCORES, npc] int. Returns (blocks, tile_sched) where
      blocks[b] = (node_lo, node_hi, cols_used)
      tile_sched[t] = (block, q, col_start)  in device execution order
    """
    npc = deg_all.shape[1]
    qcls = np.maximum(1, -(-deg_all // SLOT))          # [NCORES, npc]
    qmax = int(qcls.max())
    assert qmax * SLOT <= P, f"node degree {int(deg_all.max())} > {P}"

    # prefix counts per class for fast range queries
    pref = {}
    for q in range(1, qmax + 1):
        m = (qcls == q).astype(np.int64)
        pref[q] = np.concatenate(
            [np.zeros((NCORES, 1), np.int64), np.cumsum(m, axis=1)], axis=1)

    def cols_for(lo, hi):
        tot = 0
        per_q = {}
        for q in range(1, qmax + 1):
            cnt = pref[q][:, hi] - pref[q][:, lo]      # per core
            ntiles = int(np.max(-(-cnt // _npt(q))))
            if ntiles:
                per_q[q] = ntiles
                tot += ntiles * _npt(q)
        return tot, per_q

    blocks = []
    tile_sched = []
    lo = 0
    while lo < npc:
        hi = min(lo + BGRAN, npc)
        tot, per_q = cols_for(lo, hi)
        while hi < npc:
            nhi = min(hi + BGRAN, npc)
            t2, p2 = cols_for(lo, nhi)
            if t2 > BCOLS:
                break
            hi, tot, per_q = nhi, t2, p2
        b = len(blocks)
        cur = 0
        for q in sorted(per_q):
            for i in range(per_q[q]):
                tile_sched.append((b, q, cur))
                cur += _npt(q)
        blocks.append((lo, hi, tot))
        lo = hi
    return blocks, tile_sched


def _prep_core(x, rbf, envf, dst_local, deg, blocks, tile_sched, bf16):
    """Build one core's padded streams + node->output-position map.

    x/rbf/envf: this core's edges sorted by dst_local. envf folded: rbf
    columns are pre-multiplied by env and the bias row is env itself.
    Returns (xeg, rbg, node_pos) where node_pos[n] = flat output index.
    """
    npc = len(deg)
    TT = len(tile_sched)
    TSG = math.ceil(TT / (G * SG))
    TTg16 = TSG * SG * G

    qcls = np.maximum(1, -(-deg // SLOT))

    # tile index layout per (block, q): consecutive in schedule order
    tidx = {}
    for t, (b, q, cs) in enumerate(tile_sched):
        tidx.setdefault((b, q), []).append((t, cs))

    # block offsets in the output vector
    blk_off = np.zeros(len(blocks) + 1, dtype=np.int64)
    for b, (_, _, used) in enumerate(blocks):
        blk_off[b + 1] = blk_off[b] + used

    # assign each node a (tile, row0, col)
    node_tile = np.zeros(npc, dtype=np.int64)
    node_row0 = np.zeros(npc, dtype=np.int64)
    node_pos = np.zeros(npc, dtype=np.int64)
    for b, (nlo, nhi, _) in enumerate(blocks):
        ids = np.arange(nlo, nhi)
        for q in np.unique(qcls[nlo:nhi]):
            sel = ids[qcls[nlo:nhi] == q]
            npt = _npt(q)
            tiles = tidx[(b, int(q))]
            for j, n in enumerate(sel):
                ti, cs = tiles[j // npt]
                r = j % npt
                node_tile[n] = ti
                node_row0[n] = r * SLOT * q
                node_pos[n] = blk_off[b] + cs + r

    # per-edge slot
    starts = np.zeros(npc, dtype=np.int64)
    starts[1:] = np.cumsum(deg)[:-1]
    rank = np.arange(len(dst_local), dtype=np.int64) - starts[dst_local]
    slot = node_tile[dst_local] * P + node_row0[dst_local] + rank

    x_t = np.zeros((TTg16 * P, C), dtype=bf16)
    x_t[slot] = x
    rb_t = np.zeros((TTg16 * P, RK), dtype=bf16)
    rb_t[slot, :RBF_DIM] = rbf * envf[:, None]
    rb_t[slot, RBF_DIM] = envf

    xeg = np.zeros((TSG, P, XSGW), dtype=bf16)
    xt = x_t.reshape(TSG, SG * G, P, C)
    for u in range(SG * G):
        xeg[:, :, u * C:(u + 1) * C] = xt[:, u]
    rbg = (rb_t.reshape(TSG, SG, G, P, RK).transpose(0, 2, 4, 1, 3)
           .reshape(TSG, G * RK, SG * P))
    if FP8_FILT:
        import ml_dtypes
        fp8 = ml_dtypes.float8_e4m3fn
        # [TSG, 84, SG*P] -> [TSG, 42, SG*2P]: per group block [42, 2P]
        # cols [0:P] = rows 0:42 (tiles 0-1), cols [P:2P] = rows 42:84
        r = rbg.reshape(TSG, 2, G * RK // 2, SG, P)
        rbg = np.ascontiguousarray(
            r.transpose(0, 2, 3, 1, 4)).astype(fp8)
    return (np.ascontiguousarray(xeg), np.ascontiguousarray(rbg),
            node_pos, int(blk_off[-1]))


def prepare(x_scalar, rbf, envelop_para, edge_index_0, num_atoms,
            W_rbf, b_rbf, W1, b1, W2, b2, W3, b3):
    """Host-side sharding/layout. Returns (in_maps, meta)."""
    bf16 = _bf16()
    N = int(num_atoms)
    assert x_scalar.shape[1] == C
    assert N % NCORES == 0
    npc = N // NCORES

    dst = np.asarray(edge_index_0, dtype=np.int64)
    order = np.argsort(dst, kind="stable")
    dst_s = dst[order]
    x_s = np.asarray(x_scalar, dtype=np.float32)[order]
    rbf_s = np.asarray(rbf, dtype=np.float32)[order]
    env_s = np.asarray(envelop_para, dtype=np.float32).reshape(-1)[order]

    core_of = dst_s // npc
    core_bounds = np.searchsorted(core_of, np.arange(NCORES + 1))

    deg_all = np.zeros((NCORES, npc), dtype=np.int64)
    for c in range(NCORES):
        lo, hi = core_bounds[c], core_bounds[c + 1]
        deg_all[c] = np.bincount(dst_s[lo:hi] - c * npc, minlength=npc)

    blocks, tile_sched = _schedule(deg_all)

    # block-diagonal augmented filter weight [G*RK, G*C], bf16
    wa = np.zeros((RK, C), dtype=np.float32)
    wa[:RBF_DIM] = np.asarray(W_rbf, np.float32).T
    wa[RBF_DIM] = np.asarray(b_rbf, np.float32)
    wabd = np.zeros((G * RK, G * C), dtype=bf16)
    for j in range(G):
        wabd[j * RK:(j + 1) * RK, j * C:(j + 1) * C] = wa
    if FP8_FILT:
        import ml_dtypes
        fp8 = ml_dtypes.float8_e4m3fn
        wabd = np.stack(
            [wabd[:G * RK // 2], wabd[G * RK // 2:]], axis=1).astype(fp8)

    # constant per-class scatter matrices, packed into one [P, RCOLS]
    qs = sorted({q for _, q, _ in tile_sched})
    rq_off = {}
    cur = 0
    for q in qs:
        rq_off[q] = cur
        cur += _npt(q)
    rhs_all = np.zeros((P, cur), dtype=bf16)
    for q in qs:
        npt = _npt(q)
        for i in range(npt):
            rhs_all[i * SLOT * q:(i + 1) * SLOT * q, rq_off[q] + i] = 1.0

    consts = {
        "wabd": wabd,
        "rhs_all": rhs_all,
        "w1t": np.ascontiguousarray(np.asarray(W1, np.float32).T.astype(bf16)),
        "w2t": np.ascontiguousarray(np.asarray(W2, np.float32).T.astype(bf16)),
        "w3t": np.ascontiguousarray(np.asarray(W3, np.float32).T.astype(bf16)),
        "b1": np.asarray(b1, np.float32).reshape(C, 1),
        "b2": np.asarray(b2, np.float32).reshape(C, 1),
        "b3": np.asarray(b3, np.float32).reshape(1, 1),
    }
    in_maps = []
    perms = []
    out_len = None
    for c in range(NCORES):
        lo, hi = core_bounds[c], core_bounds[c + 1]
        xeg, rbg, node_pos, olen = _prep_core(
            x_s[lo:hi], rbf_s[lo:hi], env_s[lo:hi],
            dst_s[lo:hi] - c * npc, deg_all[c], blocks, tile_sched, bf16)
        in_maps.append({"xeg": xeg, "rbg": rbg, **consts})
        perms.append(node_pos)
        out_len = olen

    meta = dict(N=N, C=C, npc=npc, blocks=blocks, tile_sched=tile_sched,
                rq_off=rq_off, rhs_cols=cur, out_len=out_len, perms=perms)
    return in_maps, meta


# ----------------------------------------------------------- bass kernel

def build_graph(meta):
    import concourse.bacc as bacc
    import concourse.mybir as mybir
    import concourse.tile as tile

    f32 = mybir.dt.float32
    bf16 = mybir.dt.bfloat16
    AF = mybir.ActivationFunctionType
    OP = mybir.AluOpType

    blocks = meta["blocks"]
    tile_sched = meta["tile_sched"]
    rq_off = meta["rq_off"]
    RCOLS = meta["rhs_cols"]
    OUT_LEN = meta["out_len"]
    NB = len(blocks)
    TT = len(tile_sched)
    TSG = math.ceil(TT / (G * SG))
    NPAIR = math.ceil(TT / PAIR)

    blk_off = [0]
    for _, _, used in blocks:
        blk_off.append(blk_off[-1] + used)
    last_tile_of_block = {}
    for t, (b, _, _) in enumerate(tile_sched):
        last_tile_of_block[b] = t

    nc = bacc.Bacc(None, target_bir_lowering=False, debug=False)

    fp8 = mybir.dt.float8e4
    rb_dt = fp8 if FP8_FILT else bf16
    RBROWS = G * RK // 2 if FP8_FILT else G * RK

    xeg_d = nc.declare_dram_parameter("xeg", [TSG, P, XSGW], bf16,
                                      isOutput=False)
    if FP8_FILT:
        rbg_d = nc.declare_dram_parameter(
            "rbg", [TSG, RBROWS, SG, 2, P], rb_dt, isOutput=False)
        wabd_d = nc.declare_dram_parameter(
            "wabd", [RBROWS, 2, G * C], rb_dt, isOutput=False)
    else:
        rbg_d = nc.declare_dram_parameter(
            "rbg", [TSG, RBROWS, SG * P], rb_dt, isOutput=False)
        wabd_d = nc.declare_dram_parameter(
            "wabd", [RBROWS, G * C], rb_dt, isOutput=False)
    rhs_d = nc.declare_dram_parameter("rhs_all", [P, RCOLS], bf16,
                                      isOutput=False)
    w1t_d = nc.declare_dram_parameter("w1t", [C, C], bf16, isOutput=False)
    w2t_d = nc.declare_dram_parameter("w2t", [C, C], bf16, isOutput=False)
    w3t_d = nc.declare_dram_parameter("w3t", [C, 1], bf16, isOutput=False)
    b1_d = nc.declare_dram_parameter("b1", [C, 1], f32, isOutput=False)
    b2_d = nc.declare_dram_parameter("b2", [C, 1], f32, isOutput=False)
    b3_d = nc.declare_dram_parameter("b3", [1, 1], f32, isOutput=False)
    out_d = nc.declare_dram_parameter("out", [OUT_LEN], f32, isOutput=True)

    with tile.TileContext(nc) as tc:
        with (
            tc.tile_pool(name="const", bufs=1) as cp,
            tc.tile_pool(name="xin", bufs=LEADS + 3) as xp,
            tc.tile_pool(name="rin", bufs=LEADS + 3) as rp,
            tc.tile_pool(name="fc", bufs=3) as fcp,
            tc.tile_pool(name="msg", bufs=6) as mp,
            tc.tile_pool(name="mlp", bufs=2) as hp,
            tc.tile_pool(name="ys", bufs=2) as ysp,
            tc.tile_pool(name="fps", bufs=3, space="PSUM") as fps,
            tc.tile_pool(name="ops", bufs=2, space="PSUM") as ops,
        ):
            if FP8_FILT:
                wabd_s = cp.tile([RBROWS, 2, G * C], rb_dt)
                nc.sync.dma_start(out=wabd_s[:], in_=wabd_d[:, :, :])
            else:
                wabd_s = cp.tile([RBROWS, G * C], rb_dt)
                nc.sync.dma_start(out=wabd_s[:], in_=wabd_d[:, :])
            rhs_s = cp.tile([P, RCOLS], bf16)
            nc.sync.dma_start(out=rhs_s[:], in_=rhs_d[:, :])

            def load_mlp_consts():
                w1t_s = cp.tile([C, C], bf16)
                nc.sync.dma_start(out=w1t_s[:], in_=w1t_d[:, :])
                w2t_s = cp.tile([C, C], bf16)
                nc.sync.dma_start(out=w2t_s[:], in_=w2t_d[:, :])
                w3t_s = cp.tile([C, 1], bf16)
                nc.sync.dma_start(out=w3t_s[:], in_=w3t_d[:, :])
                b1_s = cp.tile([C, 1], f32)
                nc.sync.dma_start(out=b1_s[:], in_=b1_d[:, :])
                b2_s = cp.tile([C, 1], f32)
                nc.sync.dma_start(out=b2_s[:], in_=b2_d[:, :])
                b3_s = cp.tile([1, 1], f32)
                nc.sync.dma_start(out=b3_s[:], in_=b3_d[:, :])
                return w1t_s, w2t_s, w3t_s, b1_s, b2_s, b3_s

            xe_s, rb_s = {}, {}
            xe_s, rb_s = {}, {}
            msg_p = {}
            outT = {}
            pend_mlp = []

            def silu(dst_ap, src_ap, bias, wcols):
                if SILU_DECOMP:
                    z = hp.tile([C, BCOLS], f32, tag="sz")
                    nc.scalar.activation(z[:, :wcols], src_ap, AF.Identity,
                                         bias=bias[:])
                    s = hp.tile([C, BCOLS], f32, tag="ss")
                    nc.scalar.activation(s[:, :wcols], src_ap, AF.Sigmoid,
                                         bias=bias[:])
                    nc.vector.tensor_tensor(out=dst_ap, in0=z[:, :wcols],
                                            in1=s[:, :wcols], op=OP.mult)
                else:
                    nc.scalar.activation(dst_ap, src_ap, AF.Silu,
                                         bias=bias[:])

            def emit_mlp(b):
                wcols = blocks[b][2]
                o = outT.pop(b)
                a0 = hp.tile([C, BCOLS], bf16, tag="a0")
                nc.scalar.activation(a0[:, :wcols], o[:, :wcols], AF.Copy)
                h1p = ops.tile([C, BCOLS], f32, space="PSUM",
                               name="h1p", tag="outT")
                nc.tensor.matmul(out=h1p[:, :wcols], lhsT=w1t_s[:],
                                 rhs=a0[:, :wcols], start=True, stop=True)
                h1 = hp.tile([C, BCOLS], bf16, tag="h1")
                silu(h1[:, :wcols], h1p[:, :wcols], b1_s, wcols)
                h2p = ops.tile([C, BCOLS], f32, space="PSUM",
                               name="h2p", tag="outT")
                nc.tensor.matmul(out=h2p[:, :wcols], lhsT=w2t_s[:],
                                 rhs=h1[:, :wcols], start=True, stop=True)
                h2 = hp.tile([C, BCOLS], bf16, tag="h2")
                silu(h2[:, :wcols], h2p[:, :wcols], b2_s, wcols)
                yp = h2p[0:1, :]
                nc.tensor.matmul(out=yp[:, :wcols], lhsT=w3t_s[:],
                                 rhs=h2[:, :wcols], start=True, stop=True)
                ys = ysp.tile([1, BCOLS], f32, tag="ys")
                nc.scalar.activation(ys[:, :wcols], yp[:, :wcols],
                                     AF.Identity, bias=b3_s[:])
                nc.sync.dma_start(
                    out=out_d[None, blk_off[b]:blk_off[b] + wcols],
                    in_=ys[:, :wcols])

            def stage_load(s):
                xe = xp.tile([P, XSGW], bf16, tag="xe")
                nc.sync.dma_start(out=xe[:], in_=xeg_d[s, :, :])
                if FP8_FILT:
                    rb = rp.tile([RBROWS, SG, 2, P], rb_dt, tag="rb")
                    nc.sync.dma_start(out=rb[:], in_=rbg_d[s, :, :, :, :])
                else:
                    rb = rp.tile([RBROWS, SG * P], rb_dt, tag="rb")
                    nc.sync.dma_start(out=rb[:], in_=rbg_d[s, :, :])
                xe_s[s] = xe
                rb_s[s] = rb

            def stage_filt(pr):
                g0 = pr * 2
                sgi, q2 = divmod(g0, SG)
                xe = xe_s[sgi]
                rb = rb_s[sgi]
                fp2 = fps.tile([P, PAIR * C], f32, space="PSUM", tag="filt")
                for h in range(2):
                    if FP8_FILT:
                        nc.tensor.matmul(
                            out=fp2[:, h * G * C:(h + 1) * G * C],
                            lhsT=rb[:, q2 + h, :, :], rhs=wabd_s[:],
                            start=True, stop=True,
                            perf_mode=mybir.MatmulPerfMode.DoubleRow)
                    else:
                        nc.tensor.matmul(
                            out=fp2[:, h * G * C:(h + 1) * G * C],
                            lhsT=rb[:, (q2 + h) * P:(q2 + h + 1) * P],
                            rhs=wabd_s[:], start=True, stop=True)
                msg = mp.tile([P, PAIR * C], bf16, tag="msg")
                xblk = xe[:, q2 * G * C:(q2 + 2) * G * C]
                half = G * C
                # half 1 via ACT copy->bf16, half 2 via DVE straight from
                # PSUM — runs in parallel, halves the fp2 hold time
                fc = fcp.tile([P, half], bf16, tag="fc")
                nc.scalar.activation(fc[:], fp2[:, :half], AF.Copy)
                nc.vector.tensor_tensor(out=msg[:, half:],
                                        in0=fp2[:, half:],
                                        in1=xblk[:, half:], op=OP.mult)
                nc.vector.tensor_tensor(out=msg[:, :half], in0=fc[:],
                                        in1=xblk[:, :half], op=OP.mult)
                msg_p[pr] = msg

            def stage_scatter(pr):
                g0 = pr * 2
                msg = msg_p.pop(pr)
                for j in range(PAIR):
                    t = g0 * G + j
                    if t >= TT:
                        break
                    b, q, cs = tile_sched[t]
                    if b not in outT:
                        outT[b] = ops.tile([C, BCOLS], f32, space="PSUM",
                                           name="outT", tag="outT")
                    npt = _npt(q)
                    ro = rq_off[q]
                    nc.tensor.matmul(
                        out=outT[b][:, cs:cs + npt],
                        lhsT=msg[:, j * C:(j + 1) * C],
                        rhs=rhs_s[:, ro:ro + npt],
                        start=True, stop=True, skip_group_check=True)
                    if t == last_tile_of_block[b]:
                        pend_mlp.append(b)

            w1t_s, w2t_s, w3t_s, b1_s, b2_s, b3_s = load_mlp_consts()
            for si in range(min(LEADS + 1, TSG)):
                stage_load(si)
            NPAIR = math.ceil(TT / PAIR)
            for pr in range(NPAIR + LEADP):
                g0 = pr * 2
                if g0 % SG == 0:
                    s_next = g0 // SG + LEADS + 1
                    if s_next < TSG:
                        stage_load(s_next)
                if pr < NPAIR:
                    stage_filt(pr)
                if pr >= LEADP and (pr - LEADP) * PAIR < TT:
                    ps = pr - LEADP
                    stage_scatter(ps)
                    last_t = min((ps + 1) * PAIR, TT) - 1
                    cur_b = tile_sched[last_t][0]
                    while pend_mlp and (pend_mlp[0] < cur_b
                                        or (ps + 1) * PAIR >= TT):
                        emit_mlp(pend_mlp.pop(0))
            while pend_mlp:
                emit_mlp(pend_mlp.pop(0))

    nc.compile()
    return nc


# --------------------------------------------------------------- driver

def run(inputs, trace=False, tmpdir=None):
    from concourse.bass_utils import run_bass_kernel_spmd

    in_maps, meta = prepare(**inputs)
    nc = build_graph(meta)
    res = run_bass_kernel_spmd(nc, in_maps, core_ids=list(range(NCORES)),
                               trace=trace, tmpdir=tmpdir)
    npc = meta["npc"]
    outs = []
    for c in range(NCORES):
        flat = np.asarray(res.results[c]["out"])
        outs.append(flat[meta["perms"][c]])
    return np.concatenate(outs).reshape(meta["N"], 1).astype(np.float32), res


def kernel(**inputs):
    out, _ = run(inputs, trace=False)
    return out


# revision 4
# speedup vs baseline: 1.2353x; 1.0707x over previous
"""AtomWiseInvariants (GNN message passing) on 8 TRN2 NeuronCores — v5.

Strategy: shard by destination node (core i owns nodes [i*N/8, (i+1)*N/8));
host routes edges to their destination core (argsort by dst + padding), so
cores are fully independent — no collectives.

v5 eliminates the per-tile one-hot build (the DVE wall in v2-v4): each
node's edges are padded to a multiple of SLOT=8 rows, nodes are grouped by
slot-class q=ceil(d/8) and packed so a 128-row edge tile holds 16//q nodes
of one class. The scatter matmul's rhs is then a tiny CONSTANT 0/1 matrix
per class (rows 8q*i..8q*(i+1) -> column i) and each tile writes a disjoint
column range of its node block's [C, <=512] PSUM accumulator with
start=stop=True — no PSUM accumulation chains, no DVE compare op at all.
The envelope is folded into the rbf stream host-side (bias row = env), so
msg = filt * x needs just one DVE 2x multiply per 8-tile pair.

Node columns inside a block are class-permuted; the MLP is elementwise
over columns so it runs on the permuted layout, and the host inverts the
permutation when assembling the output.

Remaining per-8-tile-pair device work: 2 bf16 filter matmuls (PSUM bank
pair), 1 ACT copy f32->bf16 (or DVE multiply straight from PSUM for a
fraction of pairs, to balance ACT vs DVE), 1 DVE 2x multiply, 8 tiny
scatter matmuls. DMA is supergrouped (16 tiles per dma_start pair).
"""

import math

import numpy as np

# ---------------------------------------------------------------- config

NCORES = 8
P = 128           # partitions / edge tile size
SLOT = 8          # slot quantum per node
C = 128
G = 4             # edge tiles per filter matmul group (one PSUM bank)
PAIR = 2 * G      # tiles per ACT/DVE processing pair (two PSUM banks)
SG = 4            # groups per DMA supergroup (16 tiles)
RBF_DIM = 20
RK = RBF_DIM + 1  # augmented contraction dim (bias row)
BCOLS = 512       # node columns per MLP block (one PSUM bank)
BGRAN = 32        # node-range granularity when packing blocks
LEADP = 3         # pair software-pipeline lead distance (filt vs scatter)
LEADS = 2         # supergroup DMA lead distance

# fp8(e4m3) filter matmul via DoubleRow (0.5 cyc/row): rbf+wabd in fp8,
# contraction 84 split 42+42 across the two row-sets
FP8_FILT = False

# CoreSim lacks Silu; True decomposes it as v*sigmoid(v) for sim runs
SILU_DECOMP = False

XSGW = SG * G * C          # xeg columns per supergroup row


def _bf16():
    import ml_dtypes
    return ml_dtypes.bfloat16


def _npt(q):
    """Nodes per 128-row tile for class q (8q rows per node)."""
    return P // (SLOT * q)


# ------------------------------------------------------------- host prep

def _schedule(deg_all):
    """Build the shared (SPMD) schedule from per-core node degrees.

    deg_all: [NCORES, npc] int. Returns (blocks, tile_sched) where
      blocks[b] = (node_lo, node_hi, cols_used)
      tile_sched[t] = (block, q, col_start)  in device execution order
    """
    npc = deg_all.shape[1]
    qcls = np.maximum(1, -(-deg_all // SLOT))          # [NCORES, npc]
    qmax = int(qcls.max())
    assert qmax * SLOT <= P, f"node degree {int(deg_all.max())} > {P}"

    # prefix counts per class for fast range queries
    pref = {}
    for q in range(1, qmax + 1):
        m = (qcls == q).astype(np.int64)
        pref[q] = np.concatenate(
            [np.zeros((NCORES, 1), np.int64), np.cumsum(m, axis=1)], axis=1)

    def cols_for(lo, hi):
        tot = 0
        per_q = {}
        for q in range(1, qmax + 1):
            cnt = pref[q][:, hi] - pref[q][:, lo]      # per core
            ntiles = int(np.max(-(-cnt // _npt(q))))
            if ntiles:
                per_q[q] = ntiles
                tot += ntiles * _npt(q)
        return tot, per_q

    blocks = []
    tile_sched = []
    lo = 0
    while lo < npc:
        hi = min(lo + BGRAN, npc)
        tot, per_q = cols_for(lo, hi)
        while hi < npc:
            nhi = min(hi + BGRAN, npc)
            t2, p2 = cols_for(lo, nhi)
            if t2 > BCOLS:
                break
            hi, tot, per_q = nhi, t2, p2
        b = len(blocks)
        cur = 0
        for q in sorted(per_q):
            for i in range(per_q[q]):
                tile_sched.append((b, q, cur))
                cur += _npt(q)
        blocks.append((lo, hi, tot))
        lo = hi
    return blocks, tile_sched


def _prep_core(x, rbf, envf, dst_local, deg, blocks, tile_sched, bf16):
    """Build one core's padded streams + node->output-position map.

    x/rbf/envf: this core's edges sorted by dst_local. envf folded: rbf
    columns are pre-multiplied by env and the bias row is env itself.
    Returns (xeg, rbg, node_pos) where node_pos[n] = flat output index.
    """
    npc = len(deg)
    TT = len(tile_sched)
    TSG = math.ceil(TT / (G * SG))
    TTg16 = TSG * SG * G

    qcls = np.maximum(1, -(-deg // SLOT))

    # tile index layout per (block, q): consecutive in schedule order
    tidx = {}
    for t, (b, q, cs) in enumerate(tile_sched):
        tidx.setdefault((b, q), []).append((t, cs))

    # block offsets in the output vector
    blk_off = np.zeros(len(blocks) + 1, dtype=np.int64)
    for b, (_, _, used) in enumerate(blocks):
        blk_off[b + 1] = blk_off[b] + used

    # assign each node a (tile, row0, col)
    node_tile = np.zeros(npc, dtype=np.int64)
    node_row0 = np.zeros(npc, dtype=np.int64)
    node_pos = np.zeros(npc, dtype=np.int64)
    for b, (nlo, nhi, _) in enumerate(blocks):
        ids = np.arange(nlo, nhi)
        for q in np.unique(qcls[nlo:nhi]):
            sel = ids[qcls[nlo:nhi] == q]
            npt = _npt(q)
            tiles = tidx[(b, int(q))]
            for j, n in enumerate(sel):
                ti, cs = tiles[j // npt]
                r = j % npt
                node_tile[n] = ti
                node_row0[n] = r * SLOT * q
                node_pos[n] = blk_off[b] + cs + r

    # per-edge slot
    starts = np.zeros(npc, dtype=np.int64)
    starts[1:] = np.cumsum(deg)[:-1]
    rank = np.arange(len(dst_local), dtype=np.int64) - starts[dst_local]
    slot = node_tile[dst_local] * P + node_row0[dst_local] + rank

    x_t = np.zeros((TTg16 * P, C), dtype=bf16)
    x_t[slot] = x
    rb_t = np.zeros((TTg16 * P, RK), dtype=bf16)
    rb_t[slot, :RBF_DIM] = rbf * envf[:, None]
    rb_t[slot, RBF_DIM] = envf

    xeg = np.zeros((TSG, P, XSGW), dtype=bf16)
    xt = x_t.reshape(TSG, SG * G, P, C)
    for u in range(SG * G):
        xeg[:, :, u * C:(u + 1) * C] = xt[:, u]
    rbg = (rb_t.reshape(TSG, SG, G, P, RK).transpose(0, 2, 4, 1, 3)
           .reshape(TSG, G * RK, SG * P))
    if FP8_FILT:
        import ml_dtypes
        fp8 = ml_dtypes.float8_e4m3fn
        # [TSG, 84, SG*P] -> [TSG, 42, SG*2P]: per group block [42, 2P]
        # cols [0:P] = rows 0:42 (tiles 0-1), cols [P:2P] = rows 42:84
        r = rbg.reshape(TSG, 2, G * RK // 2, SG, P)
        rbg = np.ascontiguousarray(
            r.transpose(0, 2, 3, 1, 4)).astype(fp8)
    return (np.ascontiguousarray(xeg), np.ascontiguousarray(rbg),
            node_pos, int(blk_off[-1]))


def prepare(x_scalar, rbf, envelop_para, edge_index_0, num_atoms,
            W_rbf, b_rbf, W1, b1, W2, b2, W3, b3):
    """Host-side sharding/layout. Returns (in_maps, meta)."""
    bf16 = _bf16()
    N = int(num_atoms)
    assert x_scalar.shape[1] == C
    assert N % NCORES == 0
    npc = N // NCORES

    dst = np.asarray(edge_index_0, dtype=np.int64)
    order = np.argsort(dst, kind="stable")
    dst_s = dst[order]
    x_s = np.asarray(x_scalar, dtype=np.float32)[order]
    rbf_s = np.asarray(rbf, dtype=np.float32)[order]
    env_s = np.asarray(envelop_para, dtype=np.float32).reshape(-1)[order]

    core_of = dst_s // npc
    core_bounds = np.searchsorted(core_of, np.arange(NCORES + 1))

    deg_all = np.zeros((NCORES, npc), dtype=np.int64)
    for c in range(NCORES):
        lo, hi = core_bounds[c], core_bounds[c + 1]
        deg_all[c] = np.bincount(dst_s[lo:hi] - c * npc, minlength=npc)

    blocks, tile_sched = _schedule(deg_all)

    # block-diagonal augmented filter weight [G*RK, G*C], bf16
    wa = np.zeros((RK, C), dtype=np.float32)
    wa[:RBF_DIM] = np.asarray(W_rbf, np.float32).T
    wa[RBF_DIM] = np.asarray(b_rbf, np.float32)
    wabd = np.zeros((G * RK, G * C), dtype=bf16)
    for j in range(G):
        wabd[j * RK:(j + 1) * RK, j * C:(j + 1) * C] = wa
    if FP8_FILT:
        import ml_dtypes
        fp8 = ml_dtypes.float8_e4m3fn
        wabd = np.stack(
            [wabd[:G * RK // 2], wabd[G * RK // 2:]], axis=1).astype(fp8)

    # constant per-class scatter matrices, packed into one [P, RCOLS]
    qs = sorted({q for _, q, _ in tile_sched})
    rq_off = {}
    cur = 0
    for q in qs:
        rq_off[q] = cur
        cur += _npt(q)
    rhs_all = np.zeros((P, cur), dtype=bf16)
    for q in qs:
        npt = _npt(q)
        for i in range(npt):
            rhs_all[i * SLOT * q:(i + 1) * SLOT * q, rq_off[q] + i] = 1.0

    consts = {
        "wabd": wabd,
        "rhs_all": rhs_all,
        "w1t": np.ascontiguousarray(np.asarray(W1, np.float32).T.astype(bf16)),
        "w2t": np.ascontiguousarray(np.asarray(W2, np.float32).T.astype(bf16)),
        "w3t": np.ascontiguousarray(np.asarray(W3, np.float32).T.astype(bf16)),
        "b1": np.asarray(b1, np.float32).reshape(C, 1),
        "b2": np.asarray(b2, np.float32).reshape(C, 1),
        "b3": np.asarray(b3, np.float32).reshape(1, 1),
    }
    in_maps = []
    perms = []
    out_len = None
    for c in range(NCORES):
        lo, hi = core_bounds[c], core_bounds[c + 1]
        xeg, rbg, node_pos, olen = _prep_core(
            x_s[lo:hi], rbf_s[lo:hi], env_s[lo:hi],
            dst_s[lo:hi] - c * npc, deg_all[c], blocks, tile_sched, bf16)
        in_maps.append({"xeg": xeg, "rbg": rbg, **consts})
        perms.append(node_pos)
        out_len = olen

    meta = dict(N=N, C=C, npc=npc, blocks=blocks, tile_sched=tile_sched,
                rq_off=rq_off, rhs_cols=cur, out_len=out_len, perms=perms)
    return in_maps, meta


# ----------------------------------------------------------- bass kernel

def build_graph(meta):
    import concourse.bacc as bacc
    import concourse.mybir as mybir
    import concourse.tile as tile

    f32 = mybir.dt.float32
    bf16 = mybir.dt.bfloat16
    AF = mybir.ActivationFunctionType
    OP = mybir.AluOpType

    blocks = meta["blocks"]
    tile_sched = meta["tile_sched"]
    rq_off = meta["rq_off"]
    RCOLS = meta["rhs_cols"]
    OUT_LEN = meta["out_len"]
    NB = len(blocks)
    TT = len(tile_sched)
    TSG = math.ceil(TT / (G * SG))
    NPAIR = math.ceil(TT / PAIR)

    blk_off = [0]
    for _, _, used in blocks:
        blk_off.append(blk_off[-1] + used)
    last_tile_of_block = {}
    for t, (b, _, _) in enumerate(tile_sched):
        last_tile_of_block[b] = t

    nc = bacc.Bacc(None, target_bir_lowering=False, debug=False)

    fp8 = mybir.dt.float8e4
    rb_dt = fp8 if FP8_FILT else bf16
    RBROWS = G * RK // 2 if FP8_FILT else G * RK

    xeg_d = nc.declare_dram_parameter("xeg", [TSG, P, XSGW], bf16,
                                      isOutput=False)
    if FP8_FILT:
        rbg_d = nc.declare_dram_parameter(
            "rbg", [TSG, RBROWS, SG, 2, P], rb_dt, isOutput=False)
        wabd_d = nc.declare_dram_parameter(
            "wabd", [RBROWS, 2, G * C], rb_dt, isOutput=False)
    else:
        rbg_d = nc.declare_dram_parameter(
            "rbg", [TSG, RBROWS, SG * P], rb_dt, isOutput=False)
        wabd_d = nc.declare_dram_parameter(
            "wabd", [RBROWS, G * C], rb_dt, isOutput=False)
    rhs_d = nc.declare_dram_parameter("rhs_all", [P, RCOLS], bf16,
                                      isOutput=False)
    w1t_d = nc.declare_dram_parameter("w1t", [C, C], bf16, isOutput=False)
    w2t_d = nc.declare_dram_parameter("w2t", [C, C], bf16, isOutput=False)
    w3t_d = nc.declare_dram_parameter("w3t", [C, 1], bf16, isOutput=False)
    b1_d = nc.declare_dram_parameter("b1", [C, 1], f32, isOutput=False)
    b2_d = nc.declare_dram_parameter("b2", [C, 1], f32, isOutput=False)
    b3_d = nc.declare_dram_parameter("b3", [1, 1], f32, isOutput=False)
    out_d = nc.declare_dram_parameter("out", [OUT_LEN], f32, isOutput=True)

    with tile.TileContext(nc) as tc:
        with (
            tc.tile_pool(name="const", bufs=1) as cp,
            tc.tile_pool(name="xin", bufs=LEADS + 4) as xp,
            tc.tile_pool(name="rin", bufs=LEADS + 4) as rp,
            tc.tile_pool(name="fc", bufs=5) as fcp,
            tc.tile_pool(name="msg", bufs=6) as mp,
            tc.tile_pool(name="mlp", bufs=2) as hp,
            tc.tile_pool(name="ys", bufs=2) as ysp,
            tc.tile_pool(name="fps", bufs=3, space="PSUM") as fps,
            tc.tile_pool(name="ops", bufs=2, space="PSUM") as ops,
        ):
            if FP8_FILT:
                wabd_s = cp.tile([RBROWS, 2, G * C], rb_dt)
                nc.sync.dma_start(out=wabd_s[:], in_=wabd_d[:, :, :])
            else:
                wabd_s = cp.tile([RBROWS, G * C], rb_dt)
                nc.sync.dma_start(out=wabd_s[:], in_=wabd_d[:, :])
            rhs_s = cp.tile([P, RCOLS], bf16)
            nc.sync.dma_start(out=rhs_s[:], in_=rhs_d[:, :])

            def load_mlp_consts():
                w1t_s = cp.tile([C, C], bf16)
                nc.sync.dma_start(out=w1t_s[:], in_=w1t_d[:, :])
                w2t_s = cp.tile([C, C], bf16)
                nc.sync.dma_start(out=w2t_s[:], in_=w2t_d[:, :])
                w3t_s = cp.tile([C, 1], bf16)
                nc.sync.dma_start(out=w3t_s[:], in_=w3t_d[:, :])
                b1_s = cp.tile([C, 1], f32)
                nc.sync.dma_start(out=b1_s[:], in_=b1_d[:, :])
                b2_s = cp.tile([C, 1], f32)
                nc.sync.dma_start(out=b2_s[:], in_=b2_d[:, :])
                b3_s = cp.tile([1, 1], f32)
                nc.sync.dma_start(out=b3_s[:], in_=b3_d[:, :])
                return w1t_s, w2t_s, w3t_s, b1_s, b2_s, b3_s

            xe_s, rb_s = {}, {}
            xe_s, rb_s = {}, {}
            msg_p = {}
            outT = {}
            pend_mlp = []

            def silu(dst_ap, src_ap, bias, wcols):
                if SILU_DECOMP:
                    z = hp.tile([C, BCOLS], f32, tag="sz")
                    nc.scalar.activation(z[:, :wcols], src_ap, AF.Identity,
                                         bias=bias[:])
                    s = hp.tile([C, BCOLS], f32, tag="ss")
                    nc.scalar.activation(s[:, :wcols], src_ap, AF.Sigmoid,
                                         bias=bias[:])
                    nc.vector.tensor_tensor(out=dst_ap, in0=z[:, :wcols],
                                            in1=s[:, :wcols], op=OP.mult)
                else:
                    nc.scalar.activation(dst_ap, src_ap, AF.Silu,
                                         bias=bias[:])

            def emit_mlp(b):
                wcols = blocks[b][2]
                o = outT.pop(b)
                a0 = hp.tile([C, BCOLS], bf16, tag="a0")
                nc.scalar.activation(a0[:, :wcols], o[:, :wcols], AF.Copy)
                h1p = ops.tile([C, BCOLS], f32, space="PSUM",
                               name="h1p", tag="outT")
                nc.tensor.matmul(out=h1p[:, :wcols], lhsT=w1t_s[:],
                                 rhs=a0[:, :wcols], start=True, stop=True)
                h1 = hp.tile([C, BCOLS], bf16, tag="h1")
                silu(h1[:, :wcols], h1p[:, :wcols], b1_s, wcols)
                h2p = ops.tile([C, BCOLS], f32, space="PSUM",
                               name="h2p", tag="outT")
                nc.tensor.matmul(out=h2p[:, :wcols], lhsT=w2t_s[:],
                                 rhs=h1[:, :wcols], start=True, stop=True)
                h2 = hp.tile([C, BCOLS], bf16, tag="h2")
                silu(h2[:, :wcols], h2p[:, :wcols], b2_s, wcols)
                yp = h2p[0:1, :]
                nc.tensor.matmul(out=yp[:, :wcols], lhsT=w3t_s[:],
                                 rhs=h2[:, :wcols], start=True, stop=True)
                ys = ysp.tile([1, BCOLS], f32, tag="ys")
                nc.scalar.activation(ys[:, :wcols], yp[:, :wcols],
                                     AF.Identity, bias=b3_s[:])
                nc.sync.dma_start(
                    out=out_d[None, blk_off[b]:blk_off[b] + wcols],
                    in_=ys[:, :wcols])

            def stage_load(s):
                xe = xp.tile([P, XSGW], bf16, tag="xe")
                nc.sync.dma_start(out=xe[:], in_=xeg_d[s, :, :])
                if FP8_FILT:
                    rb = rp.tile([RBROWS, SG, 2, P], rb_dt, tag="rb")
                    nc.sync.dma_start(out=rb[:], in_=rbg_d[s, :, :, :, :])
                else:
                    rb = rp.tile([RBROWS, SG * P], rb_dt, tag="rb")
                    nc.sync.dma_start(out=rb[:], in_=rbg_d[s, :, :])
                xe_s[s] = xe
                rb_s[s] = rb

            def stage_filt(pr):
                g0 = pr * 2
                sgi, q2 = divmod(g0, SG)
                xe = xe_s[sgi]
                rb = rb_s[sgi]
                fp2 = fps.tile([P, PAIR * C], f32, space="PSUM", tag="filt")
                for h in range(2):
                    if FP8_FILT:
                        nc.tensor.matmul(
                            out=fp2[:, h * G * C:(h + 1) * G * C],
                            lhsT=rb[:, q2 + h, :, :], rhs=wabd_s[:],
                            start=True, stop=True,
                            perf_mode=mybir.MatmulPerfMode.DoubleRow)
                    else:
                        nc.tensor.matmul(
                            out=fp2[:, h * G * C:(h + 1) * G * C],
                            lhsT=rb[:, (q2 + h) * P:(q2 + h + 1) * P],
                            rhs=wabd_s[:], start=True, stop=True)
                msg = mp.tile([P, PAIR * C], bf16, tag="msg")
                xblk = xe[:, q2 * G * C:(q2 + 2) * G * C]
                half = G * C
                # half 1 via ACT copy->bf16, half 2 via DVE straight from
                # PSUM — runs in parallel, halves the fp2 hold time
                fc = fcp.tile([P, half], bf16, tag="fc")
                nc.scalar.activation(fc[:], fp2[:, :half], AF.Copy)
                nc.vector.tensor_tensor(out=msg[:, half:],
                                        in0=fp2[:, half:],
                                        in1=xblk[:, half:], op=OP.mult)
                nc.vector.tensor_tensor(out=msg[:, :half], in0=fc[:],
                                        in1=xblk[:, :half], op=OP.mult)
                msg_p[pr] = msg

            def stage_scatter(pr):
                g0 = pr * 2
                msg = msg_p.pop(pr)
                for j in range(PAIR):
                    t = g0 * G + j
                    if t >= TT:
                        break
                    b, q, cs = tile_sched[t]
                    if b not in outT:
                        outT[b] = ops.tile([C, BCOLS], f32, space="PSUM",
                                           name="outT", tag="outT")
                    npt = _npt(q)
                    ro = rq_off[q]
                    nc.tensor.matmul(
                        out=outT[b][:, cs:cs + npt],
                        lhsT=msg[:, j * C:(j + 1) * C],
                        rhs=rhs_s[:, ro:ro + npt],
                        start=True, stop=True, skip_group_check=True)
                    if t == last_tile_of_block[b]:
                        pend_mlp.append(b)

            w1t_s, w2t_s, w3t_s, b1_s, b2_s, b3_s = load_mlp_consts()
            for si in range(min(LEADS + 1, TSG)):
                stage_load(si)
            NPAIR = math.ceil(TT / PAIR)
            for pr in range(NPAIR + LEADP):
                g0 = pr * 2
                if g0 % SG == 0:
                    s_next = g0 // SG + LEADS + 1
                    if s_next < TSG:
                        stage_load(s_next)
                if pr < NPAIR:
                    stage_filt(pr)
                if pr >= LEADP and (pr - LEADP) * PAIR < TT:
                    ps = pr - LEADP
                    stage_scatter(ps)
                    last_t = min((ps + 1) * PAIR, TT) - 1
                    cur_b = tile_sched[last_t][0]
                    while pend_mlp and (pend_mlp[0] < cur_b
                                        or (ps + 1) * PAIR >= TT):
                        emit_mlp(pend_mlp.pop(0))
            while pend_mlp:
                emit_mlp(pend_mlp.pop(0))

    nc.compile()
    return nc


# --------------------------------------------------------------- driver

def run(inputs, trace=False, tmpdir=None):
    from concourse.bass_utils import run_bass_kernel_spmd

    in_maps, meta = prepare(**inputs)
    nc = build_graph(meta)
    res = run_bass_kernel_spmd(nc, in_maps, core_ids=list(range(NCORES)),
                               trace=trace, tmpdir=tmpdir)
    npc = meta["npc"]
    outs = []
    for c in range(NCORES):
        flat = np.asarray(res.results[c]["out"])
        outs.append(flat[meta["perms"][c]])
    return np.concatenate(outs).reshape(meta["N"], 1).astype(np.float32), res


def kernel(**inputs):
    out, _ = run(inputs, trace=False)
    return out


# revision 5
# speedup vs baseline: 1.2633x; 1.0227x over previous
"""AtomWiseInvariants (GNN message passing) on 8 TRN2 NeuronCores — v5.

Strategy: shard by destination node (core i owns nodes [i*N/8, (i+1)*N/8));
host routes edges to their destination core (argsort by dst + padding), so
cores are fully independent — no collectives.

v5 eliminates the per-tile one-hot build (the DVE wall in v2-v4): each
node's edges are padded to a multiple of SLOT=8 rows, nodes are grouped by
slot-class q=ceil(d/8) and packed so a 128-row edge tile holds 16//q nodes
of one class. The scatter matmul's rhs is then a tiny CONSTANT 0/1 matrix
per class (rows 8q*i..8q*(i+1) -> column i) and each tile writes a disjoint
column range of its node block's [C, <=512] PSUM accumulator with
start=stop=True — no PSUM accumulation chains, no DVE compare op at all.
The envelope is folded into the rbf stream host-side (bias row = env), so
msg = filt * x needs just one DVE 2x multiply per 8-tile pair.

Node columns inside a block are class-permuted; the MLP is elementwise
over columns so it runs on the permuted layout, and the host inverts the
permutation when assembling the output.

Remaining per-8-tile-pair device work: 2 bf16 filter matmuls (PSUM bank
pair), 1 ACT copy f32->bf16 (or DVE multiply straight from PSUM for a
fraction of pairs, to balance ACT vs DVE), 1 DVE 2x multiply, 8 tiny
scatter matmuls. DMA is supergrouped (16 tiles per dma_start pair).
"""

import math

import numpy as np

# ---------------------------------------------------------------- config

NCORES = 8
P = 128           # partitions / edge tile size
SLOT = 8          # slot quantum per node
C = 128
G = 4             # edge tiles per filter matmul group (one PSUM bank)
PAIR = 2 * G      # tiles per ACT/DVE processing pair (two PSUM banks)
SG = 4            # groups per DMA supergroup (16 tiles)
RBF_DIM = 20
RK = RBF_DIM + 1  # augmented contraction dim (bias row)
BCOLS = 512       # node columns per MLP block (one PSUM bank)
BGRAN = 32        # node-range granularity when packing blocks
LEADP = 3         # pair software-pipeline lead distance (filt vs scatter)
LEADS = 2         # supergroup DMA lead distance

# fp8(e4m3) filter matmul via DoubleRow (0.5 cyc/row): rbf+wabd in fp8,
# contraction 84 split 42+42 across the two row-sets
FP8_FILT = False

# CoreSim lacks Silu; True decomposes it as v*sigmoid(v) for sim runs
SILU_DECOMP = False

XSGW = SG * G * C          # xeg columns per supergroup row


def _bf16():
    import ml_dtypes
    return ml_dtypes.bfloat16


def _npt(q):
    """Nodes per 128-row tile for class q (8q rows per node)."""
    return P // (SLOT * q)


# ------------------------------------------------------------- host prep

def _schedule(deg_all):
    """Build the shared (SPMD) schedule from per-core node degrees.

    deg_all: [NCORES, npc] int. Returns (blocks, tile_sched) where
      blocks[b] = (node_lo, node_hi, cols_used)
      tile_sched[t] = (block, q, col_start)  in device execution order
    """
    npc = deg_all.shape[1]
    qcls = np.maximum(1, -(-deg_all // SLOT))          # [NCORES, npc]
    qmax = int(qcls.max())
    assert qmax * SLOT <= P, f"node degree {int(deg_all.max())} > {P}"

    # prefix counts per class for fast range queries
    pref = {}
    for q in range(1, qmax + 1):
        m = (qcls == q).astype(np.int64)
        pref[q] = np.concatenate(
            [np.zeros((NCORES, 1), np.int64), np.cumsum(m, axis=1)], axis=1)

    def cols_for(lo, hi):
        tot = 0
        per_q = {}
        for q in range(1, qmax + 1):
            cnt = pref[q][:, hi] - pref[q][:, lo]      # per core
            ntiles = int(np.max(-(-cnt // _npt(q))))
            if ntiles:
                per_q[q] = ntiles
                tot += ntiles * _npt(q)
        return tot, per_q

    blocks = []
    tile_sched = []
    lo = 0
    while lo < npc:
        hi = min(lo + BGRAN, npc)
        tot, per_q = cols_for(lo, hi)
        while hi < npc:
            nhi = min(hi + BGRAN, npc)
            t2, p2 = cols_for(lo, nhi)
            if t2 > BCOLS:
                break
            hi, tot, per_q = nhi, t2, p2
        b = len(blocks)
        cur = 0
        for q in sorted(per_q):
            for i in range(per_q[q]):
                tile_sched.append((b, q, cur))
                cur += _npt(q)
        blocks.append((lo, hi, tot))
        lo = hi
    return blocks, tile_sched


def _prep_core(x, rbf, envf, dst_local, deg, blocks, tile_sched, bf16):
    """Build one core's padded streams + node->output-position map.

    x/rbf/envf: this core's edges sorted by dst_local. envf folded: rbf
    columns are pre-multiplied by env and the bias row is env itself.
    Returns (xeg, rbg, node_pos) where node_pos[n] = flat output index.
    """
    npc = len(deg)
    TT = len(tile_sched)
    TSG = math.ceil(TT / (G * SG))
    TTg16 = TSG * SG * G

    qcls = np.maximum(1, -(-deg // SLOT))

    # tile index layout per (block, q): consecutive in schedule order
    tidx = {}
    for t, (b, q, cs) in enumerate(tile_sched):
        tidx.setdefault((b, q), []).append((t, cs))

    # block offsets in the output vector
    blk_off = np.zeros(len(blocks) + 1, dtype=np.int64)
    for b, (_, _, used) in enumerate(blocks):
        blk_off[b + 1] = blk_off[b] + used

    # assign each node a (tile, row0, col)
    node_tile = np.zeros(npc, dtype=np.int64)
    node_row0 = np.zeros(npc, dtype=np.int64)
    node_pos = np.zeros(npc, dtype=np.int64)
    for b, (nlo, nhi, _) in enumerate(blocks):
        ids = np.arange(nlo, nhi)
        for q in np.unique(qcls[nlo:nhi]):
            sel = ids[qcls[nlo:nhi] == q]
            npt = _npt(q)
            tiles = tidx[(b, int(q))]
            for j, n in enumerate(sel):
                ti, cs = tiles[j // npt]
                r = j % npt
                node_tile[n] = ti
                node_row0[n] = r * SLOT * q
                node_pos[n] = blk_off[b] + cs + r

    # per-edge slot
    starts = np.zeros(npc, dtype=np.int64)
    starts[1:] = np.cumsum(deg)[:-1]
    rank = np.arange(len(dst_local), dtype=np.int64) - starts[dst_local]
    slot = node_tile[dst_local] * P + node_row0[dst_local] + rank

    x_t = np.zeros((TTg16 * P, C), dtype=bf16)
    x_t[slot] = x
    rb_t = np.zeros((TTg16 * P, RK), dtype=bf16)
    rb_t[slot, :RBF_DIM] = rbf * envf[:, None]
    rb_t[slot, RBF_DIM] = envf

    xeg = np.zeros((TSG, P, XSGW), dtype=bf16)
    xt = x_t.reshape(TSG, SG * G, P, C)
    for u in range(SG * G):
        xeg[:, :, u * C:(u + 1) * C] = xt[:, u]
    rbg = (rb_t.reshape(TSG, SG, G, P, RK).transpose(0, 2, 4, 1, 3)
           .reshape(TSG, G * RK, SG * P))
    if FP8_FILT:
        import ml_dtypes
        fp8 = ml_dtypes.float8_e4m3fn
        # [TSG, 84, SG*P] -> [TSG, 42, SG*2P]: per group block [42, 2P]
        # cols [0:P] = rows 0:42 (tiles 0-1), cols [P:2P] = rows 42:84
        r = rbg.reshape(TSG, 2, G * RK // 2, SG, P)
        rbg = np.ascontiguousarray(
            r.transpose(0, 2, 3, 1, 4)).astype(fp8)
    return (np.ascontiguousarray(xeg), np.ascontiguousarray(rbg),
            node_pos, int(blk_off[-1]))


def prepare(x_scalar, rbf, envelop_para, edge_index_0, num_atoms,
            W_rbf, b_rbf, W1, b1, W2, b2, W3, b3):
    """Host-side sharding/layout. Returns (in_maps, meta)."""
    bf16 = _bf16()
    N = int(num_atoms)
    assert x_scalar.shape[1] == C
    assert N % NCORES == 0
    npc = N // NCORES

    dst = np.asarray(edge_index_0, dtype=np.int64)
    order = np.argsort(dst, kind="stable")
    dst_s = dst[order]
    x_s = np.asarray(x_scalar, dtype=np.float32)[order]
    rbf_s = np.asarray(rbf, dtype=np.float32)[order]
    env_s = np.asarray(envelop_para, dtype=np.float32).reshape(-1)[order]

    core_of = dst_s // npc
    core_bounds = np.searchsorted(core_of, np.arange(NCORES + 1))

    deg_all = np.zeros((NCORES, npc), dtype=np.int64)
    for c in range(NCORES):
        lo, hi = core_bounds[c], core_bounds[c + 1]
        deg_all[c] = np.bincount(dst_s[lo:hi] - c * npc, minlength=npc)

    blocks, tile_sched = _schedule(deg_all)

    # block-diagonal augmented filter weight [G*RK, G*C], bf16
    wa = np.zeros((RK, C), dtype=np.float32)
    wa[:RBF_DIM] = np.asarray(W_rbf, np.float32).T
    wa[RBF_DIM] = np.asarray(b_rbf, np.float32)
    wabd = np.zeros((G * RK, G * C), dtype=bf16)
    for j in range(G):
        wabd[j * RK:(j + 1) * RK, j * C:(j + 1) * C] = wa
    if FP8_FILT:
        import ml_dtypes
        fp8 = ml_dtypes.float8_e4m3fn
        wabd = np.stack(
            [wabd[:G * RK // 2], wabd[G * RK // 2:]], axis=1).astype(fp8)

    # constant per-class scatter matrices, packed into one [P, RCOLS]
    qs = sorted({q for _, q, _ in tile_sched})
    rq_off = {}
    cur = 0
    for q in qs:
        rq_off[q] = cur
        cur += _npt(q)
    rhs_all = np.zeros((P, cur), dtype=bf16)
    for q in qs:
        npt = _npt(q)
        for i in range(npt):
            rhs_all[i * SLOT * q:(i + 1) * SLOT * q, rq_off[q] + i] = 1.0

    consts = {
        "wabd": wabd,
        "rhs_all": rhs_all,
        "w1t": np.ascontiguousarray(np.asarray(W1, np.float32).T.astype(bf16)),
        "w2t": np.ascontiguousarray(np.asarray(W2, np.float32).T.astype(bf16)),
        "w3t": np.ascontiguousarray(np.asarray(W3, np.float32).T.astype(bf16)),
        "b1": np.asarray(b1, np.float32).reshape(C, 1),
        "b2": np.asarray(b2, np.float32).reshape(C, 1),
        "b3": np.asarray(b3, np.float32).reshape(1, 1),
    }
    in_maps = []
    perms = []
    out_len = None
    for c in range(NCORES):
        lo, hi = core_bounds[c], core_bounds[c + 1]
        xeg, rbg, node_pos, olen = _prep_core(
            x_s[lo:hi], rbf_s[lo:hi], env_s[lo:hi],
            dst_s[lo:hi] - c * npc, deg_all[c], blocks, tile_sched, bf16)
        in_maps.append({"xeg": xeg, "rbg": rbg, **consts})
        perms.append(node_pos)
        out_len = olen

    meta = dict(N=N, C=C, npc=npc, blocks=blocks, tile_sched=tile_sched,
                rq_off=rq_off, rhs_cols=cur, out_len=out_len, perms=perms)
    return in_maps, meta


# ----------------------------------------------------------- bass kernel

def build_graph(meta):
    import concourse.bacc as bacc
    import concourse.mybir as mybir
    import concourse.tile as tile

    f32 = mybir.dt.float32
    bf16 = mybir.dt.bfloat16
    AF = mybir.ActivationFunctionType
    OP = mybir.AluOpType

    blocks = meta["blocks"]
    tile_sched = meta["tile_sched"]
    rq_off = meta["rq_off"]
    RCOLS = meta["rhs_cols"]
    OUT_LEN = meta["out_len"]
    NB = len(blocks)
    TT = len(tile_sched)
    TSG = math.ceil(TT / (G * SG))
    NPAIR = math.ceil(TT / PAIR)

    blk_off = [0]
    for _, _, used in blocks:
        blk_off.append(blk_off[-1] + used)
    last_tile_of_block = {}
    for t, (b, _, _) in enumerate(tile_sched):
        last_tile_of_block[b] = t

    nc = bacc.Bacc(None, target_bir_lowering=False, debug=False)

    fp8 = mybir.dt.float8e4
    rb_dt = fp8 if FP8_FILT else bf16
    RBROWS = G * RK // 2 if FP8_FILT else G * RK

    xeg_d = nc.declare_dram_parameter("xeg", [TSG, P, XSGW], bf16,
                                      isOutput=False)
    if FP8_FILT:
        rbg_d = nc.declare_dram_parameter(
            "rbg", [TSG, RBROWS, SG, 2, P], rb_dt, isOutput=False)
        wabd_d = nc.declare_dram_parameter(
            "wabd", [RBROWS, 2, G * C], rb_dt, isOutput=False)
    else:
        rbg_d = nc.declare_dram_parameter(
            "rbg", [TSG, RBROWS, SG * P], rb_dt, isOutput=False)
        wabd_d = nc.declare_dram_parameter(
            "wabd", [RBROWS, G * C], rb_dt, isOutput=False)
    rhs_d = nc.declare_dram_parameter("rhs_all", [P, RCOLS], bf16,
                                      isOutput=False)
    w1t_d = nc.declare_dram_parameter("w1t", [C, C], bf16, isOutput=False)
    w2t_d = nc.declare_dram_parameter("w2t", [C, C], bf16, isOutput=False)
    w3t_d = nc.declare_dram_parameter("w3t", [C, 1], bf16, isOutput=False)
    b1_d = nc.declare_dram_parameter("b1", [C, 1], f32, isOutput=False)
    b2_d = nc.declare_dram_parameter("b2", [C, 1], f32, isOutput=False)
    b3_d = nc.declare_dram_parameter("b3", [1, 1], f32, isOutput=False)
    out_d = nc.declare_dram_parameter("out", [OUT_LEN], f32, isOutput=True)

    with tile.TileContext(nc) as tc:
        with (
            tc.tile_pool(name="const", bufs=1) as cp,
            tc.tile_pool(name="xin", bufs=LEADS + 4) as xp,
            tc.tile_pool(name="rin", bufs=LEADS + 4) as rp,
            tc.tile_pool(name="fc", bufs=5) as fcp,
            tc.tile_pool(name="msg", bufs=6) as mp,
            tc.tile_pool(name="mlp", bufs=3) as hp,
            tc.tile_pool(name="ys", bufs=2) as ysp,
            tc.tile_pool(name="fps", bufs=3, space="PSUM") as fps,
            tc.tile_pool(name="ops", bufs=2, space="PSUM") as ops,
        ):
            if FP8_FILT:
                wabd_s = cp.tile([RBROWS, 2, G * C], rb_dt)
                nc.sync.dma_start(out=wabd_s[:], in_=wabd_d[:, :, :])
            else:
                wabd_s = cp.tile([RBROWS, G * C], rb_dt)
                nc.sync.dma_start(out=wabd_s[:], in_=wabd_d[:, :])
            rhs_s = cp.tile([P, RCOLS], bf16)
            nc.sync.dma_start(out=rhs_s[:], in_=rhs_d[:, :])

            def load_mlp_consts():
                w1t_s = cp.tile([C, C], bf16)
                nc.sync.dma_start(out=w1t_s[:], in_=w1t_d[:, :])
                w2t_s = cp.tile([C, C], bf16)
                nc.sync.dma_start(out=w2t_s[:], in_=w2t_d[:, :])
                w3t_s = cp.tile([C, 1], bf16)
                nc.sync.dma_start(out=w3t_s[:], in_=w3t_d[:, :])
                b1_s = cp.tile([C, 1], f32)
                nc.sync.dma_start(out=b1_s[:], in_=b1_d[:, :])
                b2_s = cp.tile([C, 1], f32)
                nc.sync.dma_start(out=b2_s[:], in_=b2_d[:, :])
                b3_s = cp.tile([1, 1], f32)
                nc.sync.dma_start(out=b3_s[:], in_=b3_d[:, :])
                return w1t_s, w2t_s, w3t_s, b1_s, b2_s, b3_s

            xe_s, rb_s = {}, {}
            xe_s, rb_s = {}, {}
            msg_p = {}
            outT = {}
            pend_mlp = []

            def silu(dst_ap, src_ap, bias, wcols):
                if SILU_DECOMP:
                    z = hp.tile([C, BCOLS], f32, tag="sz")
                    nc.scalar.activation(z[:, :wcols], src_ap, AF.Identity,
                                         bias=bias[:])
                    s = hp.tile([C, BCOLS], f32, tag="ss")
                    nc.scalar.activation(s[:, :wcols], src_ap, AF.Sigmoid,
                                         bias=bias[:])
                    nc.vector.tensor_tensor(out=dst_ap, in0=z[:, :wcols],
                                            in1=s[:, :wcols], op=OP.mult)
                else:
                    nc.scalar.activation(dst_ap, src_ap, AF.Silu,
                                         bias=bias[:])

            mlp_q = []

            def mlp_step(force=False):
                """Advance the head pending block's MLP by one stage so PE
                sees scatters between MLP matmuls (no ACT->PE ping-pong)."""
                if not mlp_q:
                    return
                st = mlp_q[0]
                b = st["b"]
                wcols = blocks[b][2]
                stage = st["stage"]
                if stage == 0:
                    o = outT.pop(b)
                    a0 = hp.tile([C, BCOLS], bf16, tag="a0")
                    nc.scalar.activation(a0[:, :wcols], o[:, :wcols],
                                         AF.Copy)
                    st["a0"] = a0
                elif stage == 1:
                    h1p = ops.tile([C, BCOLS], f32, space="PSUM",
                                   name="h1p", tag="outT")
                    nc.tensor.matmul(out=h1p[:, :wcols], lhsT=w1t_s[:],
                                     rhs=st.pop("a0")[:, :wcols],
                                     start=True, stop=True)
                    h1 = hp.tile([C, BCOLS], bf16, tag="h1")
                    silu(h1[:, :wcols], h1p[:, :wcols], b1_s, wcols)
                    st["h1"] = h1
                elif stage == 2:
                    h2p = ops.tile([C, BCOLS], f32, space="PSUM",
                                   name="h2p", tag="outT")
                    nc.tensor.matmul(out=h2p[:, :wcols], lhsT=w2t_s[:],
                                     rhs=st.pop("h1")[:, :wcols],
                                     start=True, stop=True)
                    h2 = hp.tile([C, BCOLS], bf16, tag="h2")
                    silu(h2[:, :wcols], h2p[:, :wcols], b2_s, wcols)
                    st["h2"] = h2
                    st["h2p"] = h2p
                else:
                    h2p = st.pop("h2p")
                    yp = h2p[0:1, :]
                    nc.tensor.matmul(out=yp[:, :wcols], lhsT=w3t_s[:],
                                     rhs=st.pop("h2")[:, :wcols],
                                     start=True, stop=True)
                    ys = ysp.tile([1, BCOLS], f32, tag="ys")
                    nc.scalar.activation(ys[:, :wcols], yp[:, :wcols],
                                         AF.Identity, bias=b3_s[:])
                    nc.sync.dma_start(
                        out=out_d[None, blk_off[b]:blk_off[b] + wcols],
                        in_=ys[:, :wcols])
                    mlp_q.pop(0)
                    if force:
                        mlp_step(force=True)
                    return
                st["stage"] = stage + 1
                if force:
                    mlp_step(force=True)

            def stage_load(s):
                xe = xp.tile([P, XSGW], bf16, tag="xe")
                nc.sync.dma_start(out=xe[:], in_=xeg_d[s, :, :])
                if FP8_FILT:
                    rb = rp.tile([RBROWS, SG, 2, P], rb_dt, tag="rb")
                    nc.sync.dma_start(out=rb[:], in_=rbg_d[s, :, :, :, :])
                else:
                    rb = rp.tile([RBROWS, SG * P], rb_dt, tag="rb")
                    nc.sync.dma_start(out=rb[:], in_=rbg_d[s, :, :])
                xe_s[s] = xe
                rb_s[s] = rb

            def stage_filt(pr):
                g0 = pr * 2
                sgi, q2 = divmod(g0, SG)
                xe = xe_s[sgi]
                rb = rb_s[sgi]
                fp2 = fps.tile([P, PAIR * C], f32, space="PSUM", tag="filt")
                for h in range(2):
                    if FP8_FILT:
                        nc.tensor.matmul(
                            out=fp2[:, h * G * C:(h + 1) * G * C],
                            lhsT=rb[:, q2 + h, :, :], rhs=wabd_s[:],
                            start=True, stop=True,
                            perf_mode=mybir.MatmulPerfMode.DoubleRow)
                    else:
                        nc.tensor.matmul(
                            out=fp2[:, h * G * C:(h + 1) * G * C],
                            lhsT=rb[:, (q2 + h) * P:(q2 + h + 1) * P],
                            rhs=wabd_s[:], start=True, stop=True)
                msg = mp.tile([P, PAIR * C], bf16, tag="msg")
                xblk = xe[:, q2 * G * C:(q2 + 2) * G * C]
                half = G * C
                # half 1 via ACT copy->bf16, half 2 via DVE straight from
                # PSUM — runs in parallel, halves the fp2 hold time
                fc = fcp.tile([P, half], bf16, tag="fc")
                nc.scalar.activation(fc[:], fp2[:, :half], AF.Copy)
                nc.vector.tensor_tensor(out=msg[:, half:],
                                        in0=fp2[:, half:],
                                        in1=xblk[:, half:], op=OP.mult)
                nc.vector.tensor_tensor(out=msg[:, :half], in0=fc[:],
                                        in1=xblk[:, :half], op=OP.mult)
                msg_p[pr] = msg

            def stage_scatter(pr):
                g0 = pr * 2
                msg = msg_p.pop(pr)
                for j in range(PAIR):
                    t = g0 * G + j
                    if t >= TT:
                        break
                    b, q, cs = tile_sched[t]
                    if b not in outT:
                        outT[b] = ops.tile([C, BCOLS], f32, space="PSUM",
                                           name="outT", tag="outT")
                    npt = _npt(q)
                    ro = rq_off[q]
                    nc.tensor.matmul(
                        out=outT[b][:, cs:cs + npt],
                        lhsT=msg[:, j * C:(j + 1) * C],
                        rhs=rhs_s[:, ro:ro + npt],
                        start=True, stop=True, skip_group_check=True)
                    if t == last_tile_of_block[b]:
                        pend_mlp.append(b)

            w1t_s, w2t_s, w3t_s, b1_s, b2_s, b3_s = load_mlp_consts()
            for si in range(min(LEADS + 1, TSG)):
                stage_load(si)
            NPAIR = math.ceil(TT / PAIR)
            for pr in range(NPAIR + LEADP):
                g0 = pr * 2
                if g0 % SG == 0:
                    s_next = g0 // SG + LEADS + 1
                    if s_next < TSG:
                        stage_load(s_next)
                if pr < NPAIR:
                    stage_filt(pr)
                if pr >= LEADP and (pr - LEADP) * PAIR < TT:
                    ps = pr - LEADP
                    stage_scatter(ps)
                    last_t = min((ps + 1) * PAIR, TT) - 1
                    cur_b = tile_sched[last_t][0]
                    while pend_mlp and (pend_mlp[0] < cur_b
                                        or (ps + 1) * PAIR >= TT):
                        mlp_q.append({"b": pend_mlp.pop(0), "stage": 0})
                    mlp_step()
            mlp_step(force=True)
            while pend_mlp:
                mlp_q.append({"b": pend_mlp.pop(0), "stage": 0})
                mlp_step(force=True)

    nc.compile()
    return nc


# --------------------------------------------------------------- driver

def run(inputs, trace=False, tmpdir=None):
    from concourse.bass_utils import run_bass_kernel_spmd

    in_maps, meta = prepare(**inputs)
    nc = build_graph(meta)
    res = run_bass_kernel_spmd(nc, in_maps, core_ids=list(range(NCORES)),
                               trace=trace, tmpdir=tmpdir)
    npc = meta["npc"]
    outs = []
    for c in range(NCORES):
        flat = np.asarray(res.results[c]["out"])
        outs.append(flat[meta["perms"][c]])
    return np.concatenate(outs).reshape(meta["N"], 1).astype(np.float32), res


def kernel(**inputs):
    out, _ = run(inputs, trace=False)
    return out
